# revision 1
# baseline (speedup 1.0000x reference)
"""Trainium2 Bass kernel for CorrelatedCategoricalsLM (GRU LM).

Sharding: data-parallel over batch across 8 NeuronCores (4 rows each).
On-chip layout is "T-layout": feature dims live on SBUF partitions, the
4*T token axis (t-major: tok = 4*t + b) lives on the free axis, so the
element-wise GRU gate math runs with all 128 lanes busy.

Matmuls run as float32r (full-rate fp32 on the PE array).
"""

import sys

sys.path.insert(0, "/opt/trn_rl_repo")

import numpy as np

B, T, V, E, H, DZ = 32, 128, 32000, 512, 512, 256
NCORES = 8
BL = B // NCORES            # local batch rows per core
G3 = 3 * H                  # gate rows (r, z, n)
EC = E // 128               # embedding feature chunks
DZC = DZ // 128             # z feature chunks
KE = (E + DZ) // 128        # rnn-input feature chunks
KH = H // 128               # hidden feature chunks
MG = G3 // 128              # gate m-tiles


def _build_nc(T_=T, V_=V, vg=10, use_f32r=True, bf_rec=False, bf_out=False, skip_bn=False):
    """Build the single-core Bass/Tile program (SPMD: same program, 8 cores)."""
    import concourse.mybir as mybir
    import concourse.tile as tile
    from concourse import bacc

    dt = mybir.dt
    f32 = dt.float32
    bf16 = dt.bfloat16
    wh_dt = bf16 if bf_rec else f32
    wo_dt = bf16 if bf_out else f32
    AF = mybir.ActivationFunctionType

    TOK = BL * T_               # tokens per core
    TOKC = TOK // 128           # token tiles
    VT = V_ // 128              # vocab tiles
    NG = VT // vg               # vocab tile groups
    assert VT % vg == 0 and TOK % 128 == 0

    mm32 = dt.float32r if use_f32r else f32

    nc = bacc.Bacc("TRN2")

    # --- DRAM I/O (per core) ---
    xi = nc.dram_tensor("xi", [128, TOK // 16], dt.int16, kind="ExternalInput")
    zwi = nc.dram_tensor("zwi", [DZ, BL + H], mm32, kind="ExternalInput")  # [z.T | W_init.T]
    Wih = nc.dram_tensor("Wih", [E + DZ, G3], mm32, kind="ExternalInput")  # W_ih.T
    Whh = nc.dram_tensor("Whh", [H, G3], wh_dt, kind="ExternalInput")     # W_hh.T
    # cst cols: [identity(128) | b_init(KH) | b_ih+b_hh[r,z](MG) | b_hh[n](KH)]
    cst = nc.dram_tensor("cst", [128, 128 + KH + MG + KH], f32, kind="ExternalInput")
    emb = nc.dram_tensor("emb", [V_, E], f32, kind="ExternalInput")
    WoT = nc.dram_tensor("WoT", [H, V_], wo_dt, kind="ExternalInput")     # W_out.T
    logT = nc.dram_tensor("logT", [V_, TOK], wo_dt, kind="ExternalOutput")

    with tile.TileContext(nc) as tc:
        with tc.tile_pool(name="hs", bufs=1) as hsp:
            # hsT[:, k, BL*t : BL*(t+1)] = h_t.T chunk k (h_0 at t=0)
            hsT = hsp.tile([128, KH, BL * (T_ + 1)], f32)
            need_hsbf = bf_rec or bf_out
            hsB = hsp.tile([128, KH, BL * (T_ + 1)], bf16, name="hsB") if need_hsbf else hsT
            rec_h = hsB if bf_rec else hsT   # rhs source for the recurrence MMs
            out_h = hsB if bf_out else hsT   # rhs source for the projection MMs

            _wo_cm = tc.tile_pool(name="wo", bufs=2)
            _st_cm = tc.tile_pool(name="st", bufs=2)
            wop = _wo_cm.__enter__()
            stp = _st_cm.__enter__()
            with (
                tc.tile_pool(name="cst", bufs=1) as cstp,
                tc.tile_pool(name="gi", bufs=1) as gip,
                tc.tile_pool(name="whh", bufs=1) as whp,
            ):
                giT = gip.tile([128, MG, TOK], f32)
                bnb = cstp.tile([128, KH, BL], f32)   # b_hh(n) broadcast over b
                whh_s = whp.tile([128, KH, G3], wh_dt)
                nc.sync.dma_start(whh_s[:, :, :], Whh.ap().rearrange("(k p) g -> p k g", p=128))

                # ---------- phase 0: gather, transposes, h0, gi ----------
                with (
                    tc.tile_pool(name="pre", bufs=1) as prep,
                    tc.tile_pool(name="psP", bufs=1, space="PSUM") as psP,
                ):
                    cst_t = prep.tile([128, 128 + KH + MG + KH], f32)
                    nc.sync.dma_start(cst_t[:, :], cst.ap()[:, :])
                    ident = cst_t[:, 0:128]
                    bi_s = cst_t[:, 128:128 + KH]
                    bg_s = cst_t[:, 128 + KH:128 + KH + MG]
                    bn_s = cst_t[:, 128 + KH + MG:128 + KH + MG + KH]
                    for j in range(BL):
                        nc.vector.tensor_copy(bnb[:, :, j], bn_s[:, :])
                    idx_t = prep.tile([128, TOK // 16], dt.int16)
                    nc.sync.dma_start(idx_t[:, :], xi.ap()[:, :])
                    zwi_t = prep.tile([128, DZC, BL + H], mm32)
                    nc.sync.dma_start(zwi_t[:, :, :], zwi.ap().rearrange("(k p) c -> p k c", p=128))
                    wih_s = prep.tile([128, KE, G3], mm32)
                    nc.sync.dma_start(wih_s[:, :, :], Wih.ap().rearrange("(k p) g -> p k g", p=128))

                    # h0 = tanh(W_init @ z.T + b_init), built directly in T-layout
                    h0p = psP.tile([128, KH * BL], f32, bufs=1)
                    for m in range(KH):
                        for k in range(DZC):
                            nc.tensor.matmul(
                                h0p[:, m * BL:(m + 1) * BL],
                                lhsT=zwi_t[:, k, BL + 128 * m:BL + 128 * (m + 1)],
                                rhs=zwi_t[:, k, 0:BL],
                                start=(k == 0),
                                stop=(k == DZC - 1),
                            )
                    for m in range(KH):
                        nc.scalar.activation(
                            hsT[:, m, 0:BL], h0p[:, m * BL:(m + 1) * BL],
                            AF.Tanh, bias=bi_s[:, m:m + 1],
                        )
                    if need_hsbf:
                        nc.vector.tensor_copy(hsB[:, :, 0:BL], hsT[:, :, 0:BL])

                    # embedding gather: xe[p, c, :] = emb[idx[c*128+p], :]
                    xe = prep.tile([128, TOKC, E], f32)
                    nc.gpsimd.dma_gather(
                        out_ap=xe[:, :, :],
                        in_ap=emb.ap()[:, :],
                        idxs_ap=idx_t[:, :],
                        num_idxs=TOK,
                        num_idxs_reg=TOK,
                        elem_size=E,
                    )

                    # rnn_inT: chunks 0..EC-1 = x_embed.T, chunks EC.. = z.T repeated
                    rT = prep.tile([128, KE, TOK], mm32)
                    for hh in range(EC):
                        for c in range(TOKC):
                            tp = psP.tile([128, 128], f32, name="tp", bufs=4)
                            nc.tensor.transpose(
                                tp[:, :], xe[:, c, 128 * hh:128 * (hh + 1)], ident
                            )
                            nc.vector.tensor_copy(rT[:, hh, 128 * c:128 * (c + 1)], tp[:, :])
                    nc.vector.tensor_copy(rT[:, EC:KE, 0:BL], zwi_t[:, :, 0:BL])
                    w = BL
                    while w < TOK:
                        nc.vector.tensor_copy(rT[:, EC:KE, w:2 * w], rT[:, EC:KE, 0:w])
                        w *= 2

                    # giT = W_ih @ rnn_in.T + (b_ih + b_hh[r,z])
                    for m in range(MG):
                        pg = psP.tile([128, TOK], f32, name="pg", bufs=2)
                        for k in range(KE):
                            nc.tensor.matmul(
                                pg[:, :],
                                lhsT=wih_s[:, k, 128 * m:128 * (m + 1)],
                                rhs=rT[:, k, :],
                                start=(k == 0),
                                stop=(k == KE - 1),
                            )
                        nc.vector.tensor_scalar_add(giT[:, m, :], pg[:, :], bg_s[:, m:m + 1])

                # ---------- phase 1: GRU recurrence ----------
                with (
                    tc.tile_pool(name="psR", bufs=2, space="PSUM") as psR,
                    tc.tile_pool(name="recs", bufs=2) as recs,
                ):
                    for t in range(T_):
                        c0, c1 = BL * t, BL * (t + 1)
                        ph_rz = psR.tile([128, 8, BL], f32, name="ph_rz")
                        ph_n = psR.tile([128, KH, BL], f32, name="ph_n")
                        for m in range(MG):
                            out = ph_rz[:, m, :] if m < 8 else ph_n[:, m - 8, :]
                            for k in range(KH):
                                nc.tensor.matmul(
                                    out,
                                    lhsT=whh_s[:, k, 128 * m:128 * (m + 1)],
                                    rhs=rec_h[:, k, c0:c1],
                                    start=(k == 0),
                                    stop=(k == KH - 1),
                                )
                        a_rz = recs.tile([128, 8, BL], f32, name="a_rz")
                        nc.vector.tensor_add(a_rz[:, :, :], ph_rz[:, :, :], giT[:, 0:8, c0:c1])
                        rz = recs.tile([128, 8, BL], f32, name="rz")
                        nc.scalar.activation(rz[:, :, :], a_rz[:, :, :], AF.Sigmoid)
                        t1 = recs.tile([128, KH, BL], f32, name="t1")
                        if skip_bn:
                            nc.vector.tensor_mul(t1[:, :, :], rz[:, 0:4, :], ph_n[:, :, :])
                        else:
                            hn = recs.tile([128, KH, BL], f32, name="hn")
                            nc.vector.tensor_add(hn[:, :, :], ph_n[:, :, :], bnb[:, :, :])
                            nc.vector.tensor_mul(t1[:, :, :], rz[:, 0:4, :], hn[:, :, :])
                        t2 = recs.tile([128, KH, BL], f32, name="t2")
                        nc.vector.tensor_add(t2[:, :, :], t1[:, :, :], giT[:, 8:12, c0:c1])
                        nn = recs.tile([128, KH, BL], f32, name="nn")
                        nc.scalar.activation(nn[:, :, :], t2[:, :, :], AF.Tanh)
                        d = recs.tile([128, KH, BL], f32, name="d")
                        nc.vector.tensor_sub(d[:, :, :], hsT[:, :, c0:c1], nn[:, :, :])
                        e = recs.tile([128, KH, BL], f32, name="e")
                        nc.vector.tensor_mul(e[:, :, :], rz[:, 4:8, :], d[:, :, :])
                        if need_hsbf:
                            # bf16 state write feeds the next step's matmuls
                            nc.vector.tensor_add(hsB[:, :, c1:c1 + BL], nn[:, :, :], e[:, :, :])
                        # fp32 state (for the h_prev - n term) off the critical path
                        nc.vector.tensor_add(hsT[:, :, c1:c1 + BL], nn[:, :, :], e[:, :, :])

        # ---------- phase 2: vocab projection (logitsT = W_out @ hs.T) ----------
            WoT_r = WoT.ap().rearrange("(k p) (g j) -> g p k j", p=128, j=vg * 128)
            logT_r = logT.ap().rearrange("(g vl p) t -> g p vl t", p=128, vl=vg)
            with tc.tile_pool(name="psV", bufs=4, space="PSUM") as psV:
                for g in range(NG):
                    wg = wop.tile([128, KH, vg * 128], wo_dt, name="wg")
                    nc.sync.dma_start(wg[:, :, :], WoT_r[g])
                    st = stp.tile([128, vg, TOK], wo_dt, name="st")
                    for vl in range(vg):
                        pv = psV.tile([128, TOK], f32, name="pv")
                        for k in range(KH):
                            nc.tensor.matmul(
                                pv[:, :],
                                lhsT=wg[:, k, 128 * vl:128 * (vl + 1)],
                                rhs=out_h[:, k, BL:BL * (T_ + 1)],
                                start=(k == 0),
                                stop=(k == KH - 1),
                            )
                        if vl % 2 == 0:
                            nc.vector.tensor_copy(st[:, vl, :], pv[:, :])
                        else:
                            nc.scalar.copy(st[:, vl, :], pv[:, :])
                    nc.sync.dma_start(logT_r[g], st[:, :, :])
                _st_cm.__exit__(None, None, None)
                _wo_cm.__exit__(None, None, None)

    nc.compile()
    return nc


def _prep_core_inputs(x, z, emb, W_init, b_init, W_ih, W_hh, b_ih, b_hh, W_out,
                      T_=T, V_=V, bf_rec=False, bf_out=False):
    """Host-side prep: shard over batch, transpose weights, wrap indices."""
    import ml_dtypes

    f32 = np.float32
    bf = ml_dtypes.bfloat16
    WiT = np.ascontiguousarray(W_init.T, dtype=f32)
    WihT = np.ascontiguousarray(W_ih.T, dtype=f32)
    WhhT = np.ascontiguousarray(W_hh.T).astype(bf if bf_rec else f32)
    WoT = np.ascontiguousarray(W_out.T).astype(bf if bf_out else f32)
    embf = np.ascontiguousarray(emb, dtype=f32)
    bi_c = np.ascontiguousarray(b_init.reshape(KH, 128).T, dtype=f32)
    bg_c = np.ascontiguousarray(b_ih.reshape(MG, 128).T, dtype=f32).copy()
    bhh_c = np.ascontiguousarray(b_hh.reshape(MG, 128).T, dtype=f32)
    bg_c[:, 0:8] += bhh_c[:, 0:8]
    bn_c = np.ascontiguousarray(bhh_c[:, 8:12], dtype=f32)
    cst_c = np.ascontiguousarray(
        np.concatenate([np.eye(128, dtype=f32), bi_c, bg_c, bn_c], axis=1))

    in_maps = []
    ncores = x.shape[0] // BL
    for c in range(ncores):
        xl = x[c * BL:(c + 1) * BL]          # [BL, T]
        zl = z[c * BL:(c + 1) * BL]          # [BL, DZ]
        xs = np.ascontiguousarray(xl.T).reshape(-1)      # t-major: tok = BL*t + b
        xi16 = np.ascontiguousarray(np.tile(xs.reshape(-1, 16).T.astype(np.int16), (8, 1)))
        in_maps.append({
            "xi": xi16,
            "zwi": np.ascontiguousarray(
                np.concatenate([zl.T.astype(f32), WiT], axis=1)),
            "Wih": WihT, "Whh": WhhT, "cst": cst_c,
            "emb": embf, "WoT": WoT,
        })
    return in_maps


def _assemble_output(results, T_=T, V_=V):
    outs = []
    for res in results:
        lt = np.asarray(res["logT"]).astype(np.float32)   # [V, BL*T] tok-major cols
        lg = np.ascontiguousarray(lt.T).reshape(T_, BL, V_).transpose(1, 0, 2)
        outs.append(lg)
    return np.ascontiguousarray(np.concatenate(outs, axis=0), dtype=np.float32)


_NC_CACHE = {}


BF_REC = True    # bf16 W_hh + h in the recurrence matmuls (gates stay fp32)
BF_OUT = True    # bf16 W_out + hs in the vocab projection


def kernel(x, z, emb, W_init, b_init, W_ih, W_hh, b_ih, b_hh, W_out,
           _trace=False):
    from concourse.bass_utils import run_bass_kernel_spmd

    x = np.asarray(x)
    skip_bn = not np.asarray(b_hh)[2 * H:].any()
    key = ("full", BF_REC, BF_OUT, skip_bn)
    if key not in _NC_CACHE:
        _NC_CACHE[key] = _build_nc(bf_rec=BF_REC, bf_out=BF_OUT, skip_bn=skip_bn)
    nc = _NC_CACHE[key]
    in_maps = _prep_core_inputs(
        x, np.asarray(z), np.asarray(emb), np.asarray(W_init), np.asarray(b_init),
        np.asarray(W_ih), np.asarray(W_hh), np.asarray(b_ih), np.asarray(b_hh),
        np.asarray(W_out), bf_rec=BF_REC, bf_out=BF_OUT,
    )
    res = run_bass_kernel_spmd(
        nc, in_maps, core_ids=list(range(NCORES)), trace=_trace,
    )
    out = _assemble_output(res.results)
    if _trace:
        return out, res
    return out



# revision 6
# speedup vs baseline: 1.1269x; 1.1269x over previous
"""Trainium2 Bass kernel for CorrelatedCategoricalsLM (GRU LM).

Sharding: data-parallel over batch across 8 NeuronCores (4 rows each).
T-layout: feature dims on SBUF partitions, the 4*T token axis (t-major:
tok = 4*t + b) on the free axis.

Key optimizations over the v1 baseline:
 - gi (input-gate preactivations) folded into PSUM via identity-lhsT
   matmuls, removing a DVE add from the recurrence critical path.
 - sigma computed over [r, z, 1-z] in one activation (host supplies
   negated W_hz / gi_z copies), so the blend h' = z*h + (1-z)*n needs
   only two DVE ops after tanh.
 - h state kept in bf16 only.
 - Vocab projection interleaved into the recurrence's idle PE time:
   35 groups of 4 vocab tiles (~17.9MB bf16) stay resident in SBUF and
   project 128-col chunks as tokens complete; the remaining 28 groups
   stream during a PE-bound tail.
"""

import sys

sys.path.insert(0, "/opt/trn_rl_repo")

import numpy as np

B, T, V, E, H, DZ = 32, 128, 32000, 512, 512, 256
NCORES = 8
BL = B // NCORES            # local batch rows per core
TOK = BL * T                # tokens per core (512)
TOKC = TOK // 128
G4 = 4 * H                  # gate rows (r, z, w=-z, n)
EC = E // 128               # embedding feature chunks (4)
DZC = DZ // 128             # z feature chunks (2)
KE = (E + DZ) // 128        # rnn-input feature chunks (6)
KH = H // 128               # hidden feature chunks (4)
MG = G4 // 128              # gate m-tiles (16); 0-11 = r,z,w ; 12-15 = n

VT = V // 128               # vocab tiles (250)
GSZ = 4                     # vocab tiles per projection group
NGRP = (VT + GSZ - 1) // GSZ        # 63 groups (last has 2 tiles)
N_RES = 35                  # resident groups (140 vtiles, ~17.9MB bf16)
CCHUNK = 128                # projection column chunk
NCH = TOK // CCHUNK         # 4 chunks


def _vt_of(g):
    return min(GSZ, VT - g * GSZ)


def _build_nc(has_bias, _abl=()):
    """_abl: timing-ablation flags (produce WRONG numerics; never set in
    production): 'nofold' drops gi fold matmuls, 'noblend' skips the
    post-tanh blend, 'noproj' drops all projection items."""
    import concourse.mybir as mybir
    import concourse.tile as tile
    from concourse import bacc

    dt = mybir.dt
    f32 = dt.float32
    bf16 = dt.bfloat16
    AF = mybir.ActivationFunctionType

    nc = bacc.Bacc("TRN2")

    # --- DRAM I/O (per core) ---
    xi = nc.dram_tensor("xi", [128, TOK // 16], dt.int16, kind="ExternalInput")
    zwi = nc.dram_tensor("zwi", [DZ, BL + H], dt.float32r, kind="ExternalInput")
    Wih = nc.dram_tensor("Wih", [E + DZ, G4], bf16, kind="ExternalInput")   # [r z -z n]
    Whh = nc.dram_tensor("Whh", [H, G4], bf16, kind="ExternalInput")        # [r z -z n]
    # cst cols: [identity(128) | b_init(KH) | bg(MG: r z -z n biases)]
    cst = nc.dram_tensor("cst", [128, 128 + KH + MG], f32, kind="ExternalInput")
    emb = nc.dram_tensor("emb", [V, E], f32, kind="ExternalInput")
    WoT = nc.dram_tensor("WoT", [H, V], bf16, kind="ExternalInput")         # W_out.T
    logT = nc.dram_tensor("logT", [V, TOK], bf16, kind="ExternalOutput")

    WoT_r = WoT.ap().rearrange("(k p) v -> p k v", p=128)

    with tile.TileContext(nc) as tc:
        with tc.tile_pool(name="glob", bufs=1) as gp:
            # persistent SBUF state
            hsB = gp.tile([128, KH, BL * (T + 1)], bf16)     # bf16 h states
            giB = gp.tile([128, MG, TOK], bf16)              # gate preacts [r z -z n]
            whhB = gp.tile([128, KH, G4], bf16)
            cst_t = gp.tile([128, 128 + KH + MG], f32)
            identB = gp.tile([128, 128], bf16)

            nc.sync.dma_start(cst_t[:, :], cst.ap()[:, :])
            ident = cst_t[:, 0:128]
            bi_s = cst_t[:, 128:128 + KH]
            bg_s = cst_t[:, 128 + KH:128 + KH + MG]
            nc.vector.tensor_copy(identB[:, :], ident)
            nc.sync.dma_start(whhB[:, :, :], Whh.ap().rearrange("(k p) g -> p k g", p=128))

            # ---------- phase 0: gather, transposes, h0, gi ----------
            with (
                tc.tile_pool(name="pre", bufs=1) as prep,
                tc.tile_pool(name="wih", bufs=2) as wihp,
                tc.tile_pool(name="psP", bufs=1, space="PSUM") as psP,
                tc.tile_pool(name="psG", bufs=2, space="PSUM") as psG,
            ):
                idx_t = prep.tile([128, TOK // 16], dt.int16)
                nc.sync.dma_start(idx_t[:, :], xi.ap()[:, :])
                zwi_t = prep.tile([128, DZC, BL + H], dt.float32r)
                nc.sync.dma_start(zwi_t[:, :, :], zwi.ap().rearrange("(k p) c -> p k c", p=128))

                # h0 = tanh(W_init @ z.T + b_init) in T-layout
                h0p = psP.tile([128, KH * BL], f32, bufs=1)
                for m in range(KH):
                    for k in range(DZC):
                        nc.tensor.matmul(
                            h0p[:, m * BL:(m + 1) * BL],
                            lhsT=zwi_t[:, k, BL + 128 * m:BL + 128 * (m + 1)],
                            rhs=zwi_t[:, k, 0:BL],
                            start=(k == 0),
                            stop=(k == DZC - 1),
                        )
                for m in range(KH):
                    nc.scalar.activation(
                        hsB[:, m, 0:BL], h0p[:, m * BL:(m + 1) * BL],
                        AF.Tanh, bias=bi_s[:, m:m + 1],
                    )

                # embedding gather: xe[p, c, :] = emb[idx[c*128+p], :]
                xe = prep.tile([128, TOKC, E], f32)
                nc.gpsimd.dma_gather(
                    out_ap=xe[:, :, :],
                    in_ap=emb.ap()[:, :],
                    idxs_ap=idx_t[:, :],
                    num_idxs=TOK,
                    num_idxs_reg=TOK,
                    elem_size=E,
                )

                # rnn_inT (bf16): chunks 0..EC-1 = x_embed.T, EC.. = z.T repeated
                rT = prep.tile([128, KE, TOK], bf16)
                for hh in range(EC):
                    for c in range(TOKC):
                        tp = psP.tile([128, 128], f32, name="tp", bufs=4)
                        nc.tensor.transpose(
                            tp[:, :], xe[:, c, 128 * hh:128 * (hh + 1)], ident
                        )
                        if (hh + c) % 2 == 0:
                            nc.vector.tensor_copy(rT[:, hh, 128 * c:128 * (c + 1)], tp[:, :])
                        else:
                            nc.scalar.copy(rT[:, hh, 128 * c:128 * (c + 1)], tp[:, :])
                nc.vector.tensor_copy(rT[:, EC:KE, 0:BL], zwi_t[:, :, 0:BL])
                w = BL
                while w < TOK:
                    nc.vector.tensor_copy(rT[:, EC:KE, w:2 * w], rT[:, EC:KE, 0:w])
                    w *= 2

                # giB = W_ih @ rnn_in.T (+ bg)  (m-groups of 4, streamed weights)
                for mg in range(4):
                    wih_s = wihp.tile([128, KE, 512], bf16, name="wih_s")
                    nc.sync.dma_start(
                        wih_s[:, :, :],
                        Wih.ap().rearrange("(k p) g -> p k g", p=128)[
                            :, :, 512 * mg:512 * (mg + 1)],
                    )
                    for mi in range(4):
                        m = 4 * mg + mi
                        pg = psG.tile([128, TOK], f32, name="pg")
                        for k in range(KE):
                            nc.tensor.matmul(
                                pg[:, :],
                                lhsT=wih_s[:, k, 128 * mi:128 * (mi + 1)],
                                rhs=rT[:, k, :],
                                start=(k == 0),
                                stop=(k == KE - 1),
                            )
                        if has_bias:
                            nc.vector.tensor_scalar_add(
                                giB[:, m, :], pg[:, :], bg_s[:, m:m + 1])
                        elif m % 2 == 0:
                            nc.vector.tensor_copy(giB[:, m, :], pg[:, :])
                        else:
                            nc.scalar.copy(giB[:, m, :], pg[:, :])

            # resident W_out: pool opened after phase-0 transients free
            with tc.tile_pool(name="wores", bufs=1) as worp:
                wores = worp.tile([128, KH, N_RES * GSZ * 128], bf16)
                for g in range(N_RES):
                    nc.sync.dma_start(
                        wores[:, :, 512 * g:512 * g + 128 * _vt_of(g)],
                        WoT_r[:, :, 512 * g:512 * g + 128 * _vt_of(g)],
                    )

                # ------ interleaved projection worklist (resident, c 0..2) --
                # chunk c cols [128c,128(c+1)) complete after step 32(c+1)-1.
                sched = {}
                avail = [32 * (c + 1) + 1 for c in range(NCH)]
                queue = {c: list(range(N_RES)) for c in range(NCH - 1)}
                for t in range(T):
                    cnt = 0
                    for c in range(NCH - 1):
                        while queue[c] and t >= avail[c] and cnt < 2:
                            sched.setdefault(t, []).append((queue[c].pop(0), c))
                            cnt += 1
                tail_items = [(g, NCH - 1) for g in range(N_RES)]

                copy_flip = [0]

                def emit_item(stp, psV, g, c, wg_tile=None):
                    nvt = _vt_of(g)
                    pv = psV.tile([128, GSZ, CCHUNK], f32, name="pv")
                    for j in range(nvt):
                        for k in range(KH):
                            if wg_tile is None:
                                lhsT = wores[:, k, 512 * g + 128 * j:512 * g + 128 * (j + 1)]
                            else:
                                lhsT = wg_tile[:, k, 128 * j:128 * (j + 1)]
                            nc.tensor.matmul(
                                pv[:, j, :],
                                lhsT=lhsT,
                                rhs=hsB[:, k, BL + CCHUNK * c:BL + CCHUNK * (c + 1)],
                                start=(k == 0),
                                stop=(k == KH - 1),
                            )
                    st = stp.tile([128, GSZ, CCHUNK], bf16, name="st")
                    if copy_flip[0] % 2 == 0:
                        nc.vector.tensor_copy(st[:, 0:nvt, :], pv[:, 0:nvt, :])
                    else:
                        nc.scalar.copy(st[:, 0:nvt, :], pv[:, 0:nvt, :])
                    copy_flip[0] += 1
                    v0 = 512 * g
                    nc.sync.dma_start(
                        logT.ap()[v0:v0 + 128 * nvt, CCHUNK * c:CCHUNK * (c + 1)]
                        .rearrange("(j p) t -> p j t", p=128),
                        st[:, 0:nvt, :],
                    )

                # ------ phase 1: GRU recurrence + interleaved projection ----
                with (
                    tc.tile_pool(name="psR", bufs=2, space="PSUM") as psR,
                    tc.tile_pool(name="psV", bufs=4, space="PSUM") as psV,
                    tc.tile_pool(name="recs", bufs=2) as recs,
                    tc.tile_pool(name="stp", bufs=4) as stp,
                ):
                    for t in range(T):
                        c0, c1 = BL * t, BL * (t + 1)
                        ph = psR.tile([128, MG, BL], f32, name="ph")
                        for m in range(MG):
                            if m < 12 and 'nofold' not in _abl:
                                nc.tensor.matmul(
                                    ph[:, m, :],
                                    lhsT=identB[:, :],
                                    rhs=giB[:, m, c0:c1],
                                    start=True,
                                    stop=False,
                                )
                            for k in range(KH):
                                nc.tensor.matmul(
                                    ph[:, m, :],
                                    lhsT=whhB[:, k, 128 * m:128 * (m + 1)],
                                    rhs=hsB[:, k, c0:c1],
                                    start=((m >= 12 or 'nofold' in _abl) and k == 0),
                                    stop=(k == KH - 1),
                                )
                        rzw = recs.tile([128, 12, BL], f32, name="rzw")
                        nc.scalar.activation(rzw[:, :, :], ph[:, 0:12, :], AF.Sigmoid)
                        t1 = recs.tile([128, KH, BL], f32, name="t1")
                        nc.vector.tensor_mul(t1[:, :, :], rzw[:, 0:4, :], ph[:, 12:16, :])
                        t2 = recs.tile([128, KH, BL], f32, name="t2")
                        nc.vector.tensor_add(t2[:, :, :], t1[:, :, :], giB[:, 12:16, c0:c1])
                        e1 = recs.tile([128, KH, BL], f32, name="e1")
                        nc.vector.tensor_mul(e1[:, :, :], rzw[:, 4:8, :], hsB[:, :, c0:c1])
                        nn = recs.tile([128, KH, BL], f32, name="nn")
                        nc.scalar.activation(nn[:, :, :], t2[:, :, :], AF.Tanh)
                        if 'noblend' in _abl:
                            nc.vector.tensor_copy(hsB[:, :, c1:c1 + BL], nn[:, :, :])
                        else:
                            m2 = recs.tile([128, KH, BL], f32, name="m2")
                            nc.vector.tensor_mul(m2[:, :, :], rzw[:, 8:12, :], nn[:, :, :])
                            nc.vector.tensor_add(hsB[:, :, c1:c1 + BL], m2[:, :, :], e1[:, :, :])

                        if 'noproj' not in _abl:
                            for (g, c) in sched.get(t, ()):
                                emit_item(stp, psV, g, c)

                    # ------ tail: resident c3 + streamed groups -------------
                    with tc.tile_pool(name="wost", bufs=3) as wop:
                        for (g, c) in tail_items:
                            emit_item(stp, psV, g, c)
                        for g in range(N_RES, NGRP):
                            nvt = _vt_of(g)
                            wg = wop.tile([128, KH, GSZ * 128], bf16, name="wg")
                            nc.sync.dma_start(
                                wg[:, :, 0:128 * nvt],
                                WoT_r[:, :, 512 * g:512 * g + 128 * nvt],
                            )
                            for c in range(NCH):
                                emit_item(stp, psV, g, c, wg_tile=wg)

    nc.compile()
    return nc


def _prep_core_inputs(x, z, emb, W_init, b_init, W_ih, W_hh, b_ih, b_hh, W_out):
    """Host-side prep: shard over batch, transpose weights, wrap indices."""
    import ml_dtypes

    f32 = np.float32
    bf = ml_dtypes.bfloat16

    def gate4(W):          # [3H, X] -> [4H, X] rows [r, z, -z, n]
        r, zz, n = W[0:H], W[H:2 * H], W[2 * H:3 * H]
        return np.concatenate([r, zz, -zz, n], axis=0)

    WiT = np.ascontiguousarray(W_init.T, dtype=f32)
    Wih4 = gate4(np.asarray(W_ih, dtype=f32))
    Whh4 = gate4(np.asarray(W_hh, dtype=f32))
    WihT = np.ascontiguousarray(Wih4.T).astype(bf)
    WhhT = np.ascontiguousarray(Whh4.T).astype(bf)
    WoTc = np.ascontiguousarray(W_out.T).astype(bf)
    embf = np.ascontiguousarray(emb, dtype=f32)
    bi_c = np.ascontiguousarray(b_init.reshape(KH, 128).T, dtype=f32)
    bsum = np.asarray(b_ih, dtype=f32) + np.asarray(b_hh, dtype=f32)
    bg4 = np.concatenate([
        bsum[0:H], bsum[H:2 * H], -bsum[H:2 * H],
        np.asarray(b_ih, dtype=f32)[2 * H:3 * H],
    ])
    bg_c = np.ascontiguousarray(bg4.reshape(MG, 128).T, dtype=f32)
    cst_c = np.ascontiguousarray(
        np.concatenate([np.eye(128, dtype=f32), bi_c, bg_c], axis=1))

    in_maps = []
    ncores = x.shape[0] // BL
    for cc in range(ncores):
        xl = x[cc * BL:(cc + 1) * BL]
        zl = z[cc * BL:(cc + 1) * BL]
        xs = np.ascontiguousarray(xl.T).reshape(-1)      # t-major
        xi16 = np.ascontiguousarray(np.tile(xs.reshape(-1, 16).T.astype(np.int16), (8, 1)))
        in_maps.append({
            "xi": xi16,
            "zwi": np.ascontiguousarray(
                np.concatenate([zl.T.astype(f32), WiT], axis=1)),
            "Wih": WihT, "Whh": WhhT, "cst": cst_c,
            "emb": embf, "WoT": WoTc,
        })
    return in_maps


def _assemble_output(results):
    outs = []
    for res in results:
        lt = np.asarray(res["logT"]).astype(np.float32)   # [V, TOK] tok-major cols
        lg = np.ascontiguousarray(lt.T).reshape(T, BL, V).transpose(1, 0, 2)
        outs.append(lg)
    return np.ascontiguousarray(np.concatenate(outs, axis=0), dtype=np.float32)


_NC_CACHE = {}


def kernel(x, z, emb, W_init, b_init, W_ih, W_hh, b_ih, b_hh, W_out,
           _trace=False):
    from concourse.bass_utils import run_bass_kernel_spmd

    x = np.asarray(x)
    assert not np.asarray(b_hh)[2 * H:].any(), "kernel assumes b_hh[n] == 0"
    has_bias = bool(np.asarray(b_ih).any() or np.asarray(b_hh).any())
    key = ("v2", has_bias)
    if key not in _NC_CACHE:
        _NC_CACHE[key] = _build_nc(has_bias)
    nc = _NC_CACHE[key]
    in_maps = _prep_core_inputs(
        x, np.asarray(z), np.asarray(emb), np.asarray(W_init), np.asarray(b_init),
        np.asarray(W_ih), np.asarray(W_hh), np.asarray(b_ih), np.asarray(b_hh),
        np.asarray(W_out),
    )
    res = run_bass_kernel_spmd(
        nc, in_maps, core_ids=list(range(NCORES)), trace=_trace,
    )
    out = _assemble_output(res.results)
    if _trace:
        return out, res
    return out


# revision 18
# speedup vs baseline: 1.3129x; 1.1651x over previous
"""Trainium2 Bass kernel for CorrelatedCategoricalsLM (GRU LM).

Sharding: data-parallel over batch across 8 NeuronCores (4 rows each).
T-layout: feature dims on SBUF partitions, the 4*T token axis (t-major:
tok = 4*t + b) on the free axis.

Key optimizations over the v1 baseline:
 - gi (input-gate preactivations) folded into PSUM via identity-lhsT
   matmuls, removing a DVE add from the recurrence critical path.
 - sigma computed over [r, z, 1-z] in one activation (host supplies
   negated W_hz / gi_z copies), so the blend h' = z*h + (1-z)*n needs
   only two DVE ops after tanh.
 - h state kept in bf16 only.
 - Vocab projection interleaved into the recurrence's idle PE time:
   35 groups of 4 vocab tiles (~17.9MB bf16) stay resident in SBUF and
   project 128-col chunks as tokens complete; the remaining 28 groups
   stream during a PE-bound tail.
"""

import sys

sys.path.insert(0, "/opt/trn_rl_repo")

import numpy as np

B, T, V, E, H, DZ = 32, 128, 32000, 512, 512, 256
NCORES = 8
BL = B // NCORES            # local batch rows per core
TOK = BL * T                # tokens per core (512)
TOKC = TOK // 128
G4 = 4 * H                  # gate rows (r, z, w=-z, n)
EC = E // 128               # embedding feature chunks (4)
DZC = DZ // 128             # z feature chunks (2)
KE = (E + DZ) // 128        # rnn-input feature chunks (6)
KH = H // 128               # hidden feature chunks (4)
MG = G4 // 128              # gate m-tiles (16); 0-11 = r,z,w ; 12-15 = n

VT = V // 128               # vocab tiles (250)
GSZ = 4                     # vocab tiles per projection group
NGRP = (VT + GSZ - 1) // GSZ        # 63 groups (last has 2 tiles)
N_RES = 35                  # resident groups (140 vtiles, ~17.9MB bf16)
CCHUNK = 128                # projection column chunk
NCH = TOK // CCHUNK         # 4 chunks


def _vt_of(g):
    return min(GSZ, VT - g * GSZ)


def _build_nc(has_bias, _abl=()):
    """_abl: timing-ablation flags (produce WRONG numerics; never set in
    production): 'nofold' drops gi fold matmuls, 'noblend' skips the
    post-tanh blend, 'noproj' drops all projection items."""
    import concourse.mybir as mybir
    import concourse.tile as tile
    from concourse import bacc

    dt = mybir.dt
    f32 = dt.float32
    bf16 = dt.bfloat16
    AF = mybir.ActivationFunctionType

    nc = bacc.Bacc("TRN2")

    # --- DRAM I/O (per core) ---
    xi = nc.dram_tensor("xi", [128, TOK // 16], dt.int16, kind="ExternalInput")
    zwi = nc.dram_tensor("zwi", [DZ, BL + H], dt.float32r, kind="ExternalInput")
    Wih = nc.dram_tensor("Wih", [E + DZ, G4], bf16, kind="ExternalInput")   # [r z -z n]
    Whh = nc.dram_tensor("Whh", [H, G4], bf16, kind="ExternalInput")        # [r z -z n]
    # cst cols: [identity(128) | b_init(KH) | bg(MG: r z -z n biases)]
    cst = nc.dram_tensor("cst", [128, 128 + KH + MG], f32, kind="ExternalInput")
    emb = nc.dram_tensor("emb", [V, E], f32, kind="ExternalInput")
    WoT = nc.dram_tensor("WoT", [H, V], bf16, kind="ExternalInput")         # W_out.T
    logT = nc.dram_tensor("logT", [V, TOK], bf16, kind="ExternalOutput")

    WoT_r = WoT.ap().rearrange("(k p) v -> p k v", p=128)

    with tile.TileContext(nc) as tc:
        with tc.tile_pool(name="glob", bufs=1) as gp:
            # persistent SBUF state
            hsB = gp.tile([128, KH, BL * (T + 1)], bf16)     # bf16 h states
            giB = gp.tile([128, MG, TOK], bf16)              # gate preacts [r z -z n]
            whhB = gp.tile([128, KH, G4], bf16)
            cst_t = gp.tile([128, 128 + KH + MG], f32)
            identB = gp.tile([128, 128], bf16)

            nc.sync.dma_start(cst_t[:, :], cst.ap()[:, :])
            ident = cst_t[:, 0:128]
            bi_s = cst_t[:, 128:128 + KH]
            bg_s = cst_t[:, 128 + KH:128 + KH + MG]
            nc.vector.tensor_copy(identB[:, :], ident)

            # ---------- phase 0: gather, transposes, h0, gi ----------
            with (
                tc.tile_pool(name="pre", bufs=1) as prep,
                tc.tile_pool(name="wih", bufs=4) as wihp,
                tc.tile_pool(name="psP", bufs=1, space="PSUM") as psP,
                tc.tile_pool(name="psG", bufs=2, space="PSUM") as psG,
            ):
                idx_t = prep.tile([128, TOK // 16], dt.int16)
                nc.sync.dma_start(idx_t[:, :], xi.ap()[:, :])
                # embedding gather first: it gates the longest phase-0 chain
                xe = prep.tile([128, TOKC, E], f32)
                nc.gpsimd.dma_gather(
                    out_ap=xe[:, :, :],
                    in_ap=emb.ap()[:, :],
                    idxs_ap=idx_t[:, :],
                    num_idxs=TOK,
                    num_idxs_reg=TOK,
                    elem_size=E,
                )
                zwi_t = prep.tile([128, DZC, BL + H], dt.float32r)
                nc.sync.dma_start(zwi_t[:, :, :], zwi.ap().rearrange("(k p) c -> p k c", p=128))
                # prefetch all W_ih m-group slices up front
                wih_tiles = []
                for mg in range(4):
                    wih_s = wihp.tile([128, KE, 512], bf16, name="wih_s")
                    nc.sync.dma_start(
                        wih_s[:, :, :],
                        Wih.ap().rearrange("(k p) g -> p k g", p=128)[
                            :, :, 512 * mg:512 * (mg + 1)],
                    )
                    wih_tiles.append(wih_s)
                nc.sync.dma_start(whhB[:, :, :], Whh.ap().rearrange("(k p) g -> p k g", p=128))

                # h0 = tanh(W_init @ z.T + b_init) in T-layout
                h0p = psP.tile([128, KH * BL], f32, bufs=1)
                for m in range(KH):
                    for k in range(DZC):
                        nc.tensor.matmul(
                            h0p[:, m * BL:(m + 1) * BL],
                            lhsT=zwi_t[:, k, BL + 128 * m:BL + 128 * (m + 1)],
                            rhs=zwi_t[:, k, 0:BL],
                            start=(k == 0),
                            stop=(k == DZC - 1),
                        )
                for m in range(KH):
                    nc.scalar.activation(
                        hsB[:, m, 0:BL], h0p[:, m * BL:(m + 1) * BL],
                        AF.Tanh, bias=bi_s[:, m:m + 1],
                    )

                # rnn_inT (bf16): chunks 0..EC-1 = x_embed.T, EC.. = z.T repeated
                rT = prep.tile([128, KE, TOK], bf16)
                nc.vector.tensor_copy(rT[:, EC:KE, 0:BL], zwi_t[:, :, 0:BL])
                w = BL
                while w < TOK:
                    nc.vector.tensor_copy(rT[:, EC:KE, w:2 * w], rT[:, EC:KE, 0:w])
                    w *= 2
                for hh in range(EC):
                    for c in range(TOKC):
                        tp = psP.tile([128, 128], f32, name="tp", bufs=4)
                        nc.tensor.transpose(
                            tp[:, :], xe[:, c, 128 * hh:128 * (hh + 1)], ident
                        )
                        if (hh + c) % 2 == 0:
                            nc.vector.tensor_copy(rT[:, hh, 128 * c:128 * (c + 1)], tp[:, :])
                        else:
                            nc.scalar.copy(rT[:, hh, 128 * c:128 * (c + 1)], tp[:, :])

                # giB = W_ih @ rnn_in.T (+ bg)  (m-groups of 4, prefetched)
                for mg in range(4):
                    wih_s = wih_tiles[mg]
                    for mi in range(4):
                        m = 4 * mg + mi
                        pg = psG.tile([128, TOK], f32, name="pg")
                        for k in range(KE):
                            nc.tensor.matmul(
                                pg[:, :],
                                lhsT=wih_s[:, k, 128 * mi:128 * (mi + 1)],
                                rhs=rT[:, k, :],
                                start=(k == 0),
                                stop=(k == KE - 1),
                            )
                        if has_bias:
                            nc.vector.tensor_scalar_add(
                                giB[:, m, :], pg[:, :], bg_s[:, m:m + 1])
                        elif m % 2 == 0:
                            nc.vector.tensor_copy(giB[:, m, :], pg[:, :])
                        else:
                            nc.scalar.copy(giB[:, m, :], pg[:, :])

            # resident W_out: pool opened after phase-0 transients free
            with tc.tile_pool(name="wores", bufs=1) as worp:
                wores = worp.tile([128, KH, N_RES * GSZ * 128], bf16)
                for g in range(N_RES):
                    nc.sync.dma_start(
                        wores[:, :, 512 * g:512 * g + 128 * _vt_of(g)],
                        WoT_r[:, :, 512 * g:512 * g + 128 * _vt_of(g)],
                    )

                # ------ interleaved projection worklist ---------------------
                # chunk c cols [128c,128(c+1)) complete after step 32(c+1)-1.
                # Window slots (max 2 items/step): resident chunks 0..2 plus
                # "pass-1" streamed groups whose weights DMA in mid-window.
                from collections import deque

                N_SA = 16          # streamed pass-1 batch A: chunks 0,1
                N_SB = 12          # streamed pass-1 batch B: chunks 0,1,2
                WG_LEAD = 4        # steps between wg DMA and first use
                sched = {}         # step -> [event]
                res_q = {c: deque(range(N_RES)) for c in range(NCH - 1)}
                stream_plan = deque()
                for i in range(N_SA):
                    stream_plan.append((63 + (i * 26) // N_SA, N_RES + i, (0, 1)))
                for i in range(N_SB):
                    stream_plan.append((89 + (i * 33) // N_SB, N_RES + N_SA + i, (0, 1, 2)))
                pend_stream = deque()
                for t in range(T):
                    cap = 2
                    while stream_plan and stream_plan[0][0] <= t:
                        s0, g, chunks = stream_plan.popleft()
                        sched.setdefault(t, []).append(("wgdma", g, None))
                        pend_stream.extend(
                            (max(s0 + WG_LEAD, 32 * (c + 1) + 1), "stream", g, c)
                            for c in chunks)
                    while cap and pend_stream and pend_stream[0][0] <= t:
                        _, kind, g, c = pend_stream.popleft()
                        sched.setdefault(t, []).append((kind, g, c))
                        cap -= 1
                    for c in range(NCH - 1):
                        while cap and res_q[c] and t >= 32 * (c + 1) + 1:
                            sched.setdefault(t, []).append(("res", res_q[c].popleft(), c))
                            cap -= 1
                # leftovers (none expected, but be safe)
                spill = [("res", g, c) for c in range(NCH - 1) for g in res_q[c]]
                spill += [(k, g, c) for _, k, g, c in pend_stream]

                wg_tiles = {}
                copy_flip = [0]
                tail_flip = [0]

                def tail_eng():
                    tail_flip[0] += 1
                    return "vector" if tail_flip[0] % 2 else "scalar"

                def emit_mm(psV, g, c, wg_tile):
                    """projection matmuls for (g, c) -> pv (PSUM)"""
                    nvt = _vt_of(g)
                    pv = psV.tile([128, GSZ, CCHUNK], f32, name="pv")
                    for j in range(nvt):
                        for k in range(KH):
                            if wg_tile is None:
                                lhsT = wores[:, k, 512 * g + 128 * j:512 * g + 128 * (j + 1)]
                            else:
                                lhsT = wg_tile[:, k, 128 * j:128 * (j + 1)]
                            nc.tensor.matmul(
                                pv[:, j, :],
                                lhsT=lhsT,
                                rhs=hsB[:, k, BL + CCHUNK * c:BL + CCHUNK * (c + 1)],
                                start=(k == 0),
                                stop=(k == KH - 1),
                            )
                    return pv

                def emit_copy(g, c, pv, st, st_off, flush, eng):
                    """evacuate pv into st at st_off; if flush, DMA st cols
                    [0, st_off+CCHUNK) to logT."""
                    nvt = _vt_of(g)
                    dst = st[:, 0:nvt, st_off:st_off + CCHUNK]
                    if eng == "vector":
                        nc.vector.tensor_copy(dst, pv[:, 0:nvt, :])
                    else:
                        nc.scalar.copy(dst, pv[:, 0:nvt, :])
                    if flush:
                        ncols = st_off + CCHUNK
                        v0 = 512 * g
                        cbase = CCHUNK * c - st_off
                        nc.sync.dma_start(
                            logT.ap()[v0:v0 + 128 * nvt, cbase:cbase + ncols]
                            .rearrange("(j p) t -> p j t", p=128),
                            st[:, 0:nvt, 0:ncols],
                        )

                def emit_single(stp, psV, g, c, wg_tile=None, eng="vector"):
                    pv = emit_mm(psV, g, c, wg_tile)
                    st = stp.tile([128, GSZ, CCHUNK], bf16, name="st")
                    emit_copy(g, c, pv, st, 0, True, eng)

                def emit_pair(stp, psV, g, cpair, wg_tile):
                    # tail-only: both chunks computed as one N=256 matmul set
                    assert cpair[1] == cpair[0] + 1
                    nvt = _vt_of(g)
                    W2 = 2 * CCHUNK
                    pv = psV.tile([128, GSZ, W2], f32, name="pv2", bufs=2)
                    for j in range(nvt):
                        for k in range(KH):
                            if wg_tile is None:
                                lhsT = wores[:, k, 512 * g + 128 * j:512 * g + 128 * (j + 1)]
                            else:
                                lhsT = wg_tile[:, k, 128 * j:128 * (j + 1)]
                            nc.tensor.matmul(
                                pv[:, j, :],
                                lhsT=lhsT,
                                rhs=hsB[:, k, BL + CCHUNK * cpair[0]:BL + CCHUNK * (cpair[1] + 1)],
                                start=(k == 0),
                                stop=(k == KH - 1),
                            )
                    st = stp.tile([128, GSZ, W2], bf16, name="st2")
                    eng = tail_eng()
                    if eng == "vector":
                        nc.vector.tensor_copy(st[:, 0:nvt, :], pv[:, 0:nvt, :])
                    else:
                        nc.scalar.copy(st[:, 0:nvt, :], pv[:, 0:nvt, :])
                    v0 = 512 * g
                    cb = CCHUNK * cpair[0]
                    nc.sync.dma_start(
                        logT.ap()[v0:v0 + 128 * nvt, cb:cb + W2]
                        .rearrange("(j p) t -> p j t", p=128),
                        st[:, 0:nvt, :],
                    )

                # ------ phase 1: GRU recurrence + interleaved projection ----
                with (
                    tc.tile_pool(name="psR", bufs=2, space="PSUM") as psR,
                    tc.tile_pool(name="psV", bufs=4, space="PSUM") as psV,
                    tc.tile_pool(name="recs", bufs=2) as recs,
                    tc.tile_pool(name="stp", bufs=4) as stp,
                    tc.tile_pool(name="wost", bufs=3) as wop,
                ):
                    for t in range(T):
                        c0, c1 = BL * t, BL * (t + 1)
                        ph = psR.tile([128, MG, BL], f32, name="ph")
                        for m in range(MG):
                            if m < 12 and 'nofold' not in _abl:
                                nc.tensor.matmul(
                                    ph[:, m, :],
                                    lhsT=identB[:, :],
                                    rhs=giB[:, m, c0:c1],
                                    start=True,
                                    stop=False,
                                )
                            for k in range(KH):
                                nc.tensor.matmul(
                                    ph[:, m, :],
                                    lhsT=whhB[:, k, 128 * m:128 * (m + 1)],
                                    rhs=hsB[:, k, c0:c1],
                                    start=((m >= 12 or 'nofold' in _abl) and k == 0),
                                    stop=(k == KH - 1),
                                )
                        step_pvs = []
                        if 'noproj' not in _abl:
                            for ev in sched.get(t, ()):
                                kind, g, c = ev
                                if kind == "wgdma":
                                    nvt = _vt_of(g)
                                    wg = wop.tile([128, KH, GSZ * 128], bf16, name="wg")
                                    nc.sync.dma_start(
                                        wg[:, :, 0:128 * nvt],
                                        WoT_r[:, :, 512 * g:512 * g + 128 * nvt],
                                    )
                                    wg_tiles[g] = wg
                                else:
                                    wt = None if kind == "res" else wg_tiles[g]
                                    step_pvs.append((g, c, emit_mm(psV, g, c, wt)))

                        def flush_item(i, eng):
                            g, c, pv = step_pvs[i]
                            st = stp.tile([128, GSZ, CCHUNK], bf16, name="st")
                            emit_copy(g, c, pv, st, 0, True, eng)

                        rzw = recs.tile([128, 12, BL], f32, name="rzw")
                        nc.scalar.activation(rzw[:, :, :], ph[:, 0:12, :], AF.Sigmoid)
                        t1 = recs.tile([128, KH, BL], f32, name="t1")
                        nc.vector.tensor_mul(t1[:, :, :], rzw[:, 0:4, :], ph[:, 12:16, :])
                        t2 = recs.tile([128, KH, BL], f32, name="t2")
                        nc.vector.tensor_add(t2[:, :, :], t1[:, :, :], giB[:, 12:16, c0:c1])
                        e1 = recs.tile([128, KH, BL], f32, name="e1")
                        nc.vector.tensor_mul(e1[:, :, :], rzw[:, 4:8, :], hsB[:, :, c0:c1])
                        nn = recs.tile([128, KH, BL], f32, name="nn")
                        nc.scalar.activation(nn[:, :, :], t2[:, :, :], AF.Tanh)
                        if len(step_pvs) > (0 if t % 2 else 1):
                            flush_item(1 if t % 2 == 0 else 0, "scalar")
                        if 'noblend' in _abl:
                            nc.vector.tensor_copy(hsB[:, :, c1:c1 + BL], nn[:, :, :])
                        else:
                            m2 = recs.tile([128, KH, BL], f32, name="m2")
                            nc.vector.tensor_mul(m2[:, :, :], rzw[:, 8:12, :], nn[:, :, :])
                            nc.vector.tensor_add(hsB[:, :, c1:c1 + BL], m2[:, :, :], e1[:, :, :])
                        if step_pvs:
                            flush_item(0 if t % 2 == 0 else (len(step_pvs) - 1), "vector")

                    # ------ tail --------------------------------------------
                    if 'noproj' not in _abl:
                        def stream_wg(g):
                            nvt = _vt_of(g)
                            wg = wop.tile([128, KH, GSZ * 128], bf16, name="wg")
                            nc.sync.dma_start(
                                wg[:, :, 0:128 * nvt],
                                WoT_r[:, :, 512 * g:512 * g + 128 * nvt],
                            )
                            return wg

                        # spilled window work (seatbelt; expected empty)
                        for (kind, g, c) in spill:
                            wt = stream_wg(g) if kind == "stream" else None
                            emit_single(stp, psV, g, c, wg_tile=wt, eng=tail_eng())
                        # resident chunk-3 first: no weight DMA needed
                        for g in range(N_RES):
                            emit_single(stp, psV, g, NCH - 1, eng=tail_eng())

                        # untouched streamed groups: all 4 chunks, paired DMAs
                        for g in range(N_RES + N_SA + N_SB, NGRP):
                            wg = stream_wg(g)
                            emit_pair(stp, psV, g, (0, 1), wg)
                            emit_pair(stp, psV, g, (2, 3), wg)
                        # batch A: chunks 2,3 remain (re-stream weights)
                        for g in range(N_RES, N_RES + N_SA):
                            wg = stream_wg(g)
                            emit_pair(stp, psV, g, (2, 3), wg)
                        # batch B: chunk 3 remains
                        for g in range(N_RES + N_SA, N_RES + N_SA + N_SB):
                            wg = stream_wg(g)
                            emit_single(stp, psV, g, NCH - 1, wg_tile=wg, eng=tail_eng())

    nc.compile()
    return nc


def _prep_core_inputs(x, z, emb, W_init, b_init, W_ih, W_hh, b_ih, b_hh, W_out):
    """Host-side prep: shard over batch, transpose weights, wrap indices."""
    import ml_dtypes

    f32 = np.float32
    bf = ml_dtypes.bfloat16

    def gate4(W):          # [3H, X] -> [4H, X] rows [r, z, -z, n]
        r, zz, n = W[0:H], W[H:2 * H], W[2 * H:3 * H]
        return np.concatenate([r, zz, -zz, n], axis=0)

    WiT = np.ascontiguousarray(W_init.T, dtype=f32)
    Wih4 = gate4(np.asarray(W_ih, dtype=f32))
    Whh4 = gate4(np.asarray(W_hh, dtype=f32))
    WihT = np.ascontiguousarray(Wih4.T).astype(bf)
    WhhT = np.ascontiguousarray(Whh4.T).astype(bf)
    WoTc = np.ascontiguousarray(W_out.T).astype(bf)
    embf = np.ascontiguousarray(emb, dtype=f32)
    bi_c = np.ascontiguousarray(b_init.reshape(KH, 128).T, dtype=f32)
    bsum = np.asarray(b_ih, dtype=f32) + np.asarray(b_hh, dtype=f32)
    bg4 = np.concatenate([
        bsum[0:H], bsum[H:2 * H], -bsum[H:2 * H],
        np.asarray(b_ih, dtype=f32)[2 * H:3 * H],
    ])
    bg_c = np.ascontiguousarray(bg4.reshape(MG, 128).T, dtype=f32)
    cst_c = np.ascontiguousarray(
        np.concatenate([np.eye(128, dtype=f32), bi_c, bg_c], axis=1))

    in_maps = []
    ncores = x.shape[0] // BL
    for cc in range(ncores):
        xl = x[cc * BL:(cc + 1) * BL]
        zl = z[cc * BL:(cc + 1) * BL]
        xs = np.ascontiguousarray(xl.T).reshape(-1)      # t-major
        xi16 = np.ascontiguousarray(np.tile(xs.reshape(-1, 16).T.astype(np.int16), (8, 1)))
        in_maps.append({
            "xi": xi16,
            "zwi": np.ascontiguousarray(
                np.concatenate([zl.T.astype(f32), WiT], axis=1)),
            "Wih": WihT, "Whh": WhhT, "cst": cst_c,
            "emb": embf, "WoT": WoTc,
        })
    return in_maps


def _assemble_output(results):
    outs = []
    for res in results:
        lt = np.asarray(res["logT"]).astype(np.float32)   # [V, TOK] tok-major cols
        lg = np.ascontiguousarray(lt.T).reshape(T, BL, V).transpose(1, 0, 2)
        outs.append(lg)
    return np.ascontiguousarray(np.concatenate(outs, axis=0), dtype=np.float32)


_NC_CACHE = {}


def kernel(x, z, emb, W_init, b_init, W_ih, W_hh, b_ih, b_hh, W_out,
           _trace=False):
    from concourse.bass_utils import run_bass_kernel_spmd

    x = np.asarray(x)
    assert not np.asarray(b_hh)[2 * H:].any(), "kernel assumes b_hh[n] == 0"
    has_bias = bool(np.asarray(b_ih).any() or np.asarray(b_hh).any())
    key = ("v2", has_bias)
    if key not in _NC_CACHE:
        _NC_CACHE[key] = _build_nc(has_bias)
    nc = _NC_CACHE[key]
    in_maps = _prep_core_inputs(
        x, np.asarray(z), np.asarray(emb), np.asarray(W_init), np.asarray(b_init),
        np.asarray(W_ih), np.asarray(W_hh), np.asarray(b_ih), np.asarray(b_hh),
        np.asarray(W_out),
    )
    res = run_bass_kernel_spmd(
        nc, in_maps, core_ids=list(range(NCORES)), trace=_trace,
    )
    out = _assemble_output(res.results)
    if _trace:
        return out, res
    return out
